# revision 6
# baseline (speedup 1.0000x reference)
"""Trainium2 Bass kernel for 2-layer GAT (nn_GAT_62182536511740).

Strategy (slot-major message passing):
  * Host (pure indexing): permute nodes so each core owns 98 blocks of 128
    destination nodes with near-uniform per-source-chunk in-degree profiles
    (profile clustering).  Per (block, src-chunk) the incoming edges are laid
    out slot-major: gather position (stripe s, partition p) holds the s-th
    edge of destination node p, so every per-edge quantity is per-partition.
  * 3 launches on 8 cores (SPMD):
      1) hext1: h1 = x@W1 plus per-node attention logits -> 512B table rows
      2) msg layer 1 (+ inline h2-table build)
      3) msg layer 2 (+ log_softmax)
    Between launches the host only concatenates / reindexes device outputs.
  * Gathers: 4-queue dma_gather (int16 indices; sources split in 4 table
    chunks), 512B rows [h(128)|es(4)|ed(4)|pad] fp16.
  * Softmax: w = m * exp(leaky_relu(es_src + ed_dst)); padded slots have
    multiplicity m=0.  Aggregation: identity-lhsT matmuls accumulate
    w-weighted rows and the denominator into PSUM per block.
"""
import math
import numpy as np

import concourse.bacc as bacc
import concourse.bass as bass
import concourse.mybir as mybir
import concourse.tile as tile
from concourse.vector_clock import ScopedClock

# ----------------------------------------------------------------------------
# problem constants (hardcoded per contract)
N_NODES = 100000
N_EDGES = 1600000
D_IN = 128
H = 4
D_HID = 32
D_OUT = 32
NEG_SLOPE = 0.2

NCORES = 8
NCHUNKS = 4
ROW = 256          # fp16 elements per table row (512 B)
MAXCALL = 8        # stripes per dma_gather call (<=1024 idxs)
F32 = mybir.dt.float32
F16 = mybir.dt.float16
I16 = mybir.dt.int16

# ----------------------------------------------------------------------------
# walrus in this toolchain rejects instructions with many sync waits; move the
# excess onto same-engine nops placed before the instruction.


def _split_waits(nc, max_waits=1):
    for bb in nc.main_func.blocks:
        insts = bb.instructions
        i = 0
        while i < len(insts):
            ins = insts[i]
            si = ins.sync_info
            if si is not None and si.on_wait and len(si.on_wait) > max_waits:
                waits = list(si.on_wait)
                keep = waits[-max_waits:]
                move = waits[: len(waits) - len(keep)]
                del si.on_wait[:]
                si.on_wait.extend(keep)
                new_nops = []
                for w in move:
                    nop = nc.engines[ins.engine].nop(nofuse=True)
                    nop_ins = nop.ins
                    emitted = nc.cur_bb.bb.instructions
                    assert emitted[-1] is nop_ins
                    emitted.pop()
                    if nop_ins.sync_info is None:
                        nop_ins.sync_info = mybir.SyncInfo(on_wait=[w], on_update=[])
                    else:
                        nop_ins.sync_info.on_wait.append(w)
                    new_nops.append(nop_ins)
                insts[i:i] = new_nops
                i += len(new_nops)
            i += 1


def _drain_and_barrier_split(self, tick_clock, wait_clock):
    nc = self.nc
    drain_inst = nc.sync.drain()
    wait_clock.add_sem_waits(
        drain_inst.ins, ScopedClock({None: tick_clock.global_clock})
    )
    si = drain_inst.ins.sync_info
    if si is not None and si.on_wait and len(si.on_wait) > 1:
        waits = list(si.on_wait)
        del si.on_wait[:]
        bb = nc.cur_bb.bb
        assert bb.instructions[-1] is drain_inst.ins
        bb.instructions.pop()
        for w in waits:
            nop = nc.sync.nop(nofuse=True)
            nsi = nop.ins.sync_info
            if nsi is None:
                nop.ins.sync_info = mybir.SyncInfo(on_wait=[w], on_update=[])
            else:
                nsi.on_wait.append(w)
        bb.instructions.append(drain_inst.ins)
    nc.all_engine_barrier()
    assert self.sems is not None
    popped = nc._tile_sem_poison_stack.pop()
    assert popped is self._sem_poison
    nc.clear_and_free_semaphores(list(self.sems.allocated().values()))
    nc.all_engine_barrier()


tile.TileContext._drain_and_barrier = _drain_and_barrier_split


# ----------------------------------------------------------------------------
# host planning (pure indexing)

def build_plan(edge, n_nodes):
    src = np.asarray(edge[0], np.int64)
    dst = np.asarray(edge[1], np.int64)

    # dedup (src,dst) -> multiplicity (also provides the pad mask mechanism)
    key = dst * n_nodes + src
    uk, counts = np.unique(key, return_counts=True)
    udst = (uk // n_nodes).astype(np.int64)
    usrc = (uk % n_nodes).astype(np.int64)
    mw = counts.astype(np.float32)
    Eu = len(uk)

    chunk_map = (np.arange(n_nodes) % NCHUNKS).astype(np.int64)

    prof = np.zeros((n_nodes, NCHUNKS), np.int32)
    np.add.at(prof, (udst, chunk_map[usrc]), 1)
    deg = prof.sum(1)

    order = np.lexsort((prof[:, 2], prof[:, 1], prof[:, 0], deg))

    nblk_tot = (n_nodes + 127) // 128
    NB = (nblk_tot + NCORES - 1) // NCORES
    NPC = NB * 128
    core_nodes = -np.ones((NCORES, NPC), np.int64)
    bi = 0
    for j in range(NB):
        for c_ in range(NCORES):
            core = c_ if (j % 2 == 0) else (NCORES - 1 - c_)
            if bi >= nblk_tot:
                continue
            blk = order[bi * 128:(bi + 1) * 128]
            core_nodes[core, j * 128:j * 128 + len(blk)] = blk
            bi += 1

    # table order: chunk-major, then (core, block, partition)
    tpos = -np.ones(n_nodes, np.int64)
    chunk_bases = np.zeros(NCHUNKS + 1, np.int64)
    t = 0
    for c in range(NCHUNKS):
        chunk_bases[c] = t
        for core in range(NCORES):
            for j in range(NB):
                blk = core_nodes[core, j * 128:(j + 1) * 128]
                sel = blk[blk >= 0]
                sel = sel[chunk_map[sel] == c]
                tpos[sel] = t + np.arange(len(sel))
                t += len(sel)
    chunk_bases[NCHUNKS] = t
    assert t == n_nodes
    perm = np.empty(n_nodes, np.int64)
    perm[tpos] = np.arange(n_nodes)

    node_core = -np.ones(n_nodes, np.int64)
    node_blk = -np.ones(n_nodes, np.int64)
    node_part = -np.ones(n_nodes, np.int64)
    for core in range(NCORES):
        cn = core_nodes[core]
        pos = np.nonzero(cn >= 0)[0]
        node_core[cn[pos]] = core
        node_blk[cn[pos]] = pos // 128
        node_part[cn[pos]] = pos % 128

    ecore = node_core[udst]
    eblk = node_blk[udst]
    epart = node_part[udst]
    echunk = chunk_map[usrc]

    # slot index within (core, blk, chunk, part)
    gkey = ((ecore * NB + eblk) * NCHUNKS + echunk) * 128 + epart
    eorder = np.lexsort((gkey,))
    gk_sorted = gkey[eorder]
    grp_start = np.r_[True, gk_sorted[1:] != gk_sorted[:-1]]
    idx_in_grp = np.arange(Eu) - np.maximum.accumulate(
        np.where(grp_start, np.arange(Eu), 0))
    eslot = np.empty(Eu, np.int64)
    eslot[eorder] = idx_in_grp

    cnt = np.zeros((NCORES, NB, NCHUNKS, 128), np.int32)
    np.add.at(cnt, (ecore, eblk, echunk, epart), 1)
    S = cnt.max(axis=(0, 3)).astype(np.int64)           # [NB, NCHUNKS] compiled

    # static call schedule: per block j, per chunk c, calls of <= MAXCALL stripes
    calls = []   # (j, c, ns, iw_off, sc_off)  sc_off = stripe offset within block
    iw = 0
    block_ts = S.sum(1)                                  # stripes per block
    for j in range(NB):
        sc = 0
        for c in range(NCHUNKS):
            s_cj = int(S[j, c])
            k = 0
            while k < s_cj:
                ns = min(MAXCALL, s_cj - k)
                calls.append((j, c, ns, iw, sc))
                iw += ns * 8          # int16 cols per call ( ns*128/16 )
                sc += ns
                k += ns
    IW = iw
    TOTS = int(block_ts.sum())

    # per-core device tables
    idx_tab = np.zeros((NCORES, 128, IW), np.int16)
    m_tab = np.zeros((NCORES, 128, TOTS), np.float32)
    gs_of_block = np.zeros(NB + 1, np.int64)
    gs_of_block[1:] = np.cumsum(block_ts)

    # slot-major flat edge arrays
    src_tpos = tpos[usrc]
    for core in range(NCORES):
        esel = np.nonzero(ecore == core)[0]
        if len(esel) == 0:
            continue
        ej, ec, ep, es_ = eblk[esel], echunk[esel], epart[esel], eslot[esel]
        # m table: global stripe = gs_of_block[j] + (chunk stripe base) + slot
        chunk_s_base = np.zeros((NB, NCHUNKS), np.int64)
        chunk_s_base[:, 1:] = np.cumsum(S, axis=1)[:, :-1]
        gstripe = gs_of_block[ej] + chunk_s_base[ej, ec] + es_
        m_tab[core, ep, gstripe] = mw[esel]
        # idx values per call
        val = (src_tpos[esel] - chunk_bases[ec]).astype(np.int64)
        # per-call local position: stripe-within-call * 128 + part
        # build flat [S_cj*128] arrays per (j,c) then slice per call
        flat_pos_in_chunkgrp = es_ * 128 + ep
        keyjc = ej * NCHUNKS + ec
        ordjc = np.lexsort((flat_pos_in_chunkgrp, keyjc))
        # iterate calls
        ptr = 0
        esel_o = esel[ordjc]
        kj, kc = ej[ordjc], ec[ordjc]
        kpos = flat_pos_in_chunkgrp[ordjc]
        kval = val[ordjc]
        # group boundaries per (j,c)
        bnd = np.r_[0, np.nonzero((kj[1:] != kj[:-1]) | (kc[1:] != kc[:-1]))[0] + 1, len(kj)]
        grp_map = {}
        for b in range(len(bnd) - 1):
            lo = bnd[b]
            grp_map[(int(kj[lo]), int(kc[lo]))] = (bnd[b], bnd[b + 1])
        for (j, c, ns, iwo, sco) in calls:
            lo_hi = grp_map.get((j, c))
            flat = np.zeros(ns * 128, np.int64)
            if lo_hi is not None:
                lo, hi = lo_hi
                # positions within this call's stripe range
                sbase = (sco - int(np.sum(S[j, :c]))) * 128  # call start within group
                p0, p1 = sbase, sbase + ns * 128
                seg = slice(lo + np.searchsorted(kpos[lo:hi], p0),
                            lo + np.searchsorted(kpos[lo:hi], p1))
                flat[kpos[seg] - p0] = kval[seg]
            wrap = flat.reshape(ns * 8, 16).T.astype(np.int16)  # [16, ns*8]
            idx_tab[core, :, iwo:iwo + ns * 8] = np.tile(wrap, (8, 1))

    return dict(
        perm=perm, tpos=tpos, core_nodes=core_nodes, chunk_bases=chunk_bases,
        NB=NB, NPC=NPC, S=S, calls=calls, IW=IW, TOTS=TOTS,
        idx_tab=idx_tab, m_tab=m_tab, gs_of_block=gs_of_block,
        n_nodes=n_nodes,
    )


# ----------------------------------------------------------------------------
# bass builders

def build_hext(seg_len):
    """Launch 1: per core computes table rows for `seg_len` nodes.

    inputs : xT [128, seg_len] fp16, Wt [128,128] fp16,
             as_rep [128,128] fp32, ad_rep [128,128] fp32
    output : hx [seg_len, ROW] fp16  rows = [h(128) | es(4) | ed(4) | junk]
    """
    nc = bacc.Bacc("TRN2", num_swdge_queues=4)
    xT = nc.dram_tensor("xT", [128, seg_len], F16, kind="ExternalInput")
    Wt = nc.dram_tensor("Wt", [128, 128], F16, kind="ExternalInput")
    as_rep = nc.dram_tensor("as_rep", [128, 128], F32, kind="ExternalInput")
    ad_rep = nc.dram_tensor("ad_rep", [128, 128], F32, kind="ExternalInput")
    hx = nc.dram_tensor("hx", [seg_len, ROW], F16, kind="ExternalOutput")

    ntiles = (seg_len + 127) // 128
    with tile.TileContext(nc) as tc:
        with (
            tc.tile_pool(name="consts", bufs=1) as cpool,
            tc.tile_pool(name="work", bufs=4) as pool,
            tc.tile_pool(name="ps", bufs=2, space="PSUM") as pp,
        ):
            wt = cpool.tile([128, 128], F16)
            nc.sync.dma_start(out=wt[:], in_=Wt[:])
            asr = cpool.tile([128, 128], F32)
            nc.sync.dma_start(out=asr[:], in_=as_rep[:])
            adr = cpool.tile([128, 128], F32)
            nc.sync.dma_start(out=adr[:], in_=ad_rep[:])
            for t in range(ntiles):
                nt = min(128, seg_len - t * 128)
                xt = pool.tile([128, 128], F16, tag="xt")
                nc.sync.dma_start(out=xt[:, :nt], in_=xT[:, t * 128:t * 128 + nt])
                ph = pp.tile([128, 128], F32)
                nc.tensor.matmul(ph[:nt, :], lhsT=xt[:, :nt], rhs=wt[:],
                                 start=True, stop=True)
                row = pool.tile([128, ROW], F16, tag="row")
                nc.vector.memset(row[:], 0.0)
                nc.vector.tensor_copy(row[:nt, 0:128], ph[:nt, :])
                scr = pool.tile([128, 32], F32, tag="scr")
                for h in range(H):
                    nc.vector.scalar_tensor_tensor(
                        out=scr[:nt, :], in0=ph[:nt, h * 32:(h + 1) * 32],
                        scalar=1.0, in1=asr[:nt, h * 32:(h + 1) * 32],
                        op0=mybir.AluOpType.mult, op1=mybir.AluOpType.mult,
                        accum_out=row[:nt, 128 + h:129 + h])
                for h in range(H):
                    nc.vector.scalar_tensor_tensor(
                        out=scr[:nt, :], in0=ph[:nt, h * 32:(h + 1) * 32],
                        scalar=1.0, in1=adr[:nt, h * 32:(h + 1) * 32],
                        op0=mybir.AluOpType.mult, op1=mybir.AluOpType.mult,
                        accum_out=row[:nt, 132 + h:133 + h])
                nc.sync.dma_start(out=hx[t * 128:t * 128 + nt, :], in_=row[:nt, :])
    nc.compile()
    _split_waits(nc, max_waits=1)
    return nc


def build_msg(plan, n_nodes, layer2):
    """Launch 2/3: slot-major message passing for one layer on each core.

    inputs : tab [n_nodes, ROW] fp16, idxs [128, IW] int16, ms [128, TOTS] fp16,
             eds [NB*128, 4] fp32, btile [128,128] fp32, ident [128,128] fp16,
             (layer1 only) W2t [128,128] fp16, a2s_rep/a2d_rep [128,128] fp32
    output : layer1: hx2 [NPC, ROW] fp16 ; layer2: outp [NPC, 128] fp32
    """
    NB, S, calls, IW, TOTS = plan["NB"], plan["S"], plan["calls"], plan["IW"], plan["TOTS"]
    NPC = plan["NPC"]
    cb = plan["chunk_bases"]
    gs_of_block = plan["gs_of_block"]

    nc = bacc.Bacc("TRN2", num_swdge_queues=4)
    tab = nc.dram_tensor("tab", [n_nodes, ROW], F16, kind="ExternalInput")
    idxs = nc.dram_tensor("idxs", [128, IW], I16, kind="ExternalInput")
    ms = nc.dram_tensor("ms", [128, TOTS], F32, kind="ExternalInput")
    eds = nc.dram_tensor("eds", [NB * 128, 4], F16, kind="ExternalInput")
    btile = nc.dram_tensor("btile", [128, 128], F32, kind="ExternalInput")
    identt = nc.dram_tensor("ident", [128, 128], F16, kind="ExternalInput")
    if not layer2:
        W2t = nc.dram_tensor("W2t", [128, 128], F16, kind="ExternalInput")
        a2s = nc.dram_tensor("a2s_rep", [128, 128], F32, kind="ExternalInput")
        a2d = nc.dram_tensor("a2d_rep", [128, 128], F32, kind="ExternalInput")
        hx2 = nc.dram_tensor("hx2", [NPC, ROW], F16, kind="ExternalOutput")
    else:
        outp = nc.dram_tensor("outp", [NPC, 128], F32, kind="ExternalOutput")

    A = mybir.AluOpType
    qn = 0
    with tile.TileContext(nc) as tc:
        with (
            tc.tile_pool(name="consts", bufs=1) as cpool,
            tc.tile_pool(name="gath", bufs=6) as gp,
            tc.tile_pool(name="ip", bufs=6) as ipool,
            tc.tile_pool(name="wp", bufs=6) as wp,
            tc.tile_pool(name="msgp", bufs=6) as mp,
            tc.tile_pool(name="blkp", bufs=3) as bp,
            tc.tile_pool(name="finp", bufs=3) as fp_,
            tc.tile_pool(name="psb", bufs=2, space="PSUM") as ppb,
            tc.tile_pool(name="psx", bufs=2, space="PSUM") as ppx,
        ):
            ident = cpool.tile([128, 128], F16)
            nc.sync.dma_start(out=ident[:], in_=identt[:])
            bt = cpool.tile([128, 128], F32)
            nc.sync.dma_start(out=bt[:], in_=btile[:])
            if not layer2:
                w2 = cpool.tile([128, 128], F16)
                nc.sync.dma_start(out=w2[:], in_=W2t[:])
                a2sr = cpool.tile([128, 128], F32)
                nc.sync.dma_start(out=a2sr[:], in_=a2s[:])
                a2dr = cpool.tile([128, 128], F32)
                nc.sync.dma_start(out=a2dr[:], in_=a2d[:])

            ci = 0  # call index
            for j in range(NB):
                TS = int(S[j].sum())
                if TS == 0:
                    continue
                mt = bp.tile([128, TS], F32, tag="mt")
                nc.sync.dma_start(
                    out=mt[:, :TS],
                    in_=ms[:, int(gs_of_block[j]):int(gs_of_block[j]) + TS])
                edt = bp.tile([128, 4], F16, tag="edt")
                nc.sync.dma_start(out=edt[:], in_=eds[j * 128:(j + 1) * 128, :])
                pb = ppb.tile([128, 132], F32)
                sc_done = 0
                while ci < len(calls) and calls[ci][0] == j:
                    _, c, ns, iwo, sco = calls[ci]
                    it = ipool.tile([128, MAXCALL * 8], I16, tag="it")
                    nc.sync.dma_start(out=it[:, :ns * 8],
                                      in_=idxs[:, iwo:iwo + ns * 8])
                    gt = gp.tile([128, MAXCALL * ROW], F16, tag="gt")
                    nc.gpsimd.dma_gather(
                        gt[:, :ns * ROW].rearrange("p (k e) -> p k e", e=ROW),
                        tab[int(cb[c]):int(cb[c + 1]), :],
                        it[:, :ns * 8], ns * 128, ns * 128, ROW,
                        queue_num=qn % 4)
                    qn += 1
                    # w path
                    wt_ = wp.tile([128, MAXCALL * 4], F32, tag="wt")
                    es_v = gt[:, :ns * ROW].rearrange("p (k e) -> p k e", e=ROW)[:, :, 128:132]
                    _edt = edt[:]
                    nc.vector.tensor_tensor(
                        out=wt_[:, :ns * 4].rearrange("p (k e) -> p k e", e=4),
                        in0=es_v,
                        in1=bass.AP(_edt.tensor, _edt.offset,
                                    [_edt.ap[0], [0, ns], [1, 4]]),
                        op=A.add)
                    nc.vector.scalar_tensor_tensor(
                        out=wt_[:, :ns * 4], in0=wt_[:, :ns * 4], scalar=NEG_SLOPE,
                        in1=wt_[:, :ns * 4], op0=A.mult, op1=A.max)
                    nc.scalar.activation(wt_[:, :ns * 4], wt_[:, :ns * 4],
                                         mybir.ActivationFunctionType.Exp)
                    wm = wp.tile([128, MAXCALL * 4], F16, tag="wm")
                    mv = mt[:, sc_done:sc_done + ns]
                    nc.vector.tensor_tensor(
                        out=wm[:, :ns * 4],
                        in0=wt_[:, :ns * 4],
                        in1=bass.AP(mv.tensor, mv.offset, [mv.ap[0], [mv.ap[1][0], ns], [0, 4]]),
                        op=A.mult)
                    # msg = [h*w | w]
                    msg = mp.tile([128, MAXCALL * 132], F16, tag="msg")
                    msg_v = msg[:, :ns * 132].rearrange("p (k e) -> p k e", e=132)
                    h_v = gt[:, :ns * ROW].rearrange("p (k e) -> p k e", e=ROW)[:, :, 0:128]
                    wm_v = wm[:, :ns * 4].rearrange("p (k e) -> p k e", e=4)
                    nc.vector.tensor_tensor(
                        out=msg_v[:, :, 0:128].rearrange("p k (h d) -> p k h d", d=32),
                        in0=h_v.rearrange("p k (h d) -> p k h d", d=32),
                        in1=bass.AP(wm.tensor, wm.offset,
                                    [wm.ap[0], [4, ns], [1, 4], [0, 32]]),
                        op=A.mult)
                    nc.vector.tensor_copy(msg_v[:, :, 128:132], wm_v[:])
                    for s in range(ns):
                        nc.tensor.matmul(
                            pb[:], lhsT=ident[:], rhs=msg[:, s * 132:(s + 1) * 132],
                            start=(sc_done + s == 0), stop=(sc_done + s == TS - 1))
                    sc_done += ns
                    ci += 1
                # finalize block j
                den = fp_.tile([128, 4], F32, tag="den")
                nc.vector.tensor_scalar_add(den[:], pb[:, 128:132], 1e-20)
                nc.vector.reciprocal(den[:], den[:])
                t1 = fp_.tile([128, 128], F32, tag="t1")
                nc.vector.tensor_tensor(
                    out=t1[:].rearrange("p (h d) -> p h d", d=32),
                    in0=pb[:, 0:128].rearrange("p (h d) -> p h d", d=32),
                    in1=bass.AP(den.tensor, den.offset, [den.ap[0], [1, 4], [0, 32]]),
                    op=A.mult)
                nc.vector.tensor_tensor(out=t1[:], in0=t1[:], in1=bt[:], op=A.add)
                if not layer2:
                    x2 = fp_.tile([128, 128], F16, tag="x2")
                    nc.vector.tensor_scalar_max(x2[:], t1[:], 0.0)
                    px = ppx.tile([128, 128], F16)
                    nc.tensor.transpose(px[:], x2[:], ident[:])
                    x2t = fp_.tile([128, 128], F16, tag="x2t")
                    nc.vector.tensor_copy(x2t[:], px[:])
                    ph2 = ppx.tile([128, 128], F32)
                    nc.tensor.matmul(ph2[:], lhsT=x2t[:], rhs=w2[:],
                                     start=True, stop=True)
                    row = fp_.tile([128, ROW], F16, tag="row")
                    nc.vector.memset(row[:], 0.0)
                    nc.vector.tensor_copy(row[:, 0:128], ph2[:])
                    scr = fp_.tile([128, 32], F32, tag="scr")
                    for h in range(H):
                        nc.vector.scalar_tensor_tensor(
                            out=scr[:], in0=ph2[:, h * 32:(h + 1) * 32], scalar=1.0,
                            in1=a2sr[:, h * 32:(h + 1) * 32],
                            op0=A.mult, op1=A.mult,
                            accum_out=row[:, 128 + h:129 + h])
                    for h in range(H):
                        nc.vector.scalar_tensor_tensor(
                            out=scr[:], in0=ph2[:, h * 32:(h + 1) * 32], scalar=1.0,
                            in1=a2dr[:, h * 32:(h + 1) * 32],
                            op0=A.mult, op1=A.mult,
                            accum_out=row[:, 132 + h:133 + h])
                    nc.sync.dma_start(out=hx2[j * 128:(j + 1) * 128, :], in_=row[:])
                else:
                    et = fp_.tile([128, 128], F32, tag="et")
                    nc.scalar.activation(et[:], t1[:],
                                         mybir.ActivationFunctionType.Exp)
                    ssum = fp_.tile([128, 1], F32, tag="ssum")
                    nc.vector.tensor_reduce(ssum[:], et[:],
                                            axis=mybir.AxisListType.X, op=A.add)
                    nc.scalar.activation(ssum[:], ssum[:],
                                         mybir.ActivationFunctionType.Ln)
                    nc.vector.tensor_scalar_mul(ssum[:], ssum[:], -1.0)
                    to = fp_.tile([128, 128], F32, tag="to")
                    nc.scalar.activation(to[:], t1[:],
                                         mybir.ActivationFunctionType.Identity,
                                         bias=ssum[:, 0:1])
                    nc.sync.dma_start(out=outp[j * 128:(j + 1) * 128, :], in_=to[:])
    nc.compile()
    _split_waits(nc, max_waits=1)
    return nc


# ----------------------------------------------------------------------------
# runner

def _rep_heads(a):
    """[H, d] -> [128, H*d] fp32 replicated across partitions."""
    return np.tile(a.reshape(1, -1).astype(np.float32), (128, 1))


def _run(nc, in_maps):
    from concourse.bass_utils import run_bass_kernel_spmd
    return run_bass_kernel_spmd(nc, in_maps, core_ids=list(range(NCORES)),
                                trace=False).results


def run_pipeline(inputs, n_nodes, run=_run):
    edge = np.asarray(inputs["edge"])
    x = np.asarray(inputs["features"], np.float32)
    W1 = np.asarray(inputs["W1"], np.float32)
    a1s = np.asarray(inputs["a1_src"], np.float32)
    a1d = np.asarray(inputs["a1_dst"], np.float32)
    b1 = np.asarray(inputs["b1"], np.float32)
    W2 = np.asarray(inputs["W2"], np.float32)
    a2s = np.asarray(inputs["a2_src"], np.float32)
    a2d = np.asarray(inputs["a2_dst"], np.float32)
    b2 = np.asarray(inputs["b2"], np.float32)

    plan = build_plan(edge, n_nodes)
    NB, NPC = plan["NB"], plan["NPC"]
    perm, tpos = plan["perm"], plan["tpos"]
    core_nodes = plan["core_nodes"]

    # ---- launch 1: hext1
    seg = n_nodes // NCORES
    assert seg * NCORES == n_nodes
    nc1 = build_hext(seg)
    in1 = []
    for core in range(NCORES):
        seg_nodes = perm[core * seg:(core + 1) * seg]
        xT = np.ascontiguousarray(x[seg_nodes].astype(np.float16).T)
        in1.append({
            "xT": xT, "Wt": W1.astype(np.float16),
            "as_rep": _rep_heads(a1s), "ad_rep": _rep_heads(a1d),
        })
    res1 = run(nc1, in1)
    tab1 = np.concatenate([np.asarray(res1[c]["hx"]) for c in range(NCORES)], 0)

    # ed per (core, block-partition) from table rows
    def ed_for(tab):
        eds = np.zeros((NCORES, NB * 128, 4), np.float16)
        for core in range(NCORES):
            cn = core_nodes[core]
            vm = cn >= 0
            eds[core][vm] = tab[tpos[cn[vm]], 132:136]
        return eds

    eds1 = ed_for(tab1)

    # ---- launch 2: layer-1 message passing + inline h2 table rows
    nc2 = build_msg(plan, n_nodes, layer2=False)
    ident = np.eye(128, dtype=np.float16)
    in2 = []
    for core in range(NCORES):
        in2.append({
            "tab": tab1, "idxs": plan["idx_tab"][core], "ms": plan["m_tab"][core],
            "eds": eds1[core], "btile": np.tile(b1.reshape(1, -1), (128, 1)).astype(np.float32),
            "ident": ident, "W2t": W2.astype(np.float16),
            "a2s_rep": _rep_heads(a2s), "a2d_rep": _rep_heads(a2d),
        })
    res2 = run(nc2, in2)

    # assemble layer-2 table (block-order rows -> table order)
    tab2 = np.zeros((n_nodes, ROW), np.float16)
    for core in range(NCORES):
        cn = core_nodes[core]
        vm = cn >= 0
        tab2[tpos[cn[vm]]] = np.asarray(res2[core]["hx2"])[vm]
    eds2 = ed_for(tab2)

    # ---- launch 3: layer-2 message passing + log_softmax
    nc3 = build_msg(plan, n_nodes, layer2=True)
    in3 = []
    for core in range(NCORES):
        in3.append({
            "tab": tab2, "idxs": plan["idx_tab"][core], "ms": plan["m_tab"][core],
            "eds": eds2[core], "btile": np.tile(b2.reshape(1, -1), (128, 1)).astype(np.float32),
            "ident": ident,
        })
    res3 = run(nc3, in3)

    out = np.zeros((n_nodes, H * D_OUT), np.float32)
    for core in range(NCORES):
        cn = core_nodes[core]
        vm = cn >= 0
        out[cn[vm]] = np.asarray(res3[core]["outp"])[vm]
    return out


def kernel(**inputs):
    return run_pipeline(inputs, N_NODES).astype(np.float32)


# revision 9
# speedup vs baseline: 1.0963x; 1.0963x over previous
"""Trainium2 Bass kernel for 2-layer GAT (nn_GAT_62182536511740).

Strategy (slot-major message passing):
  * Host (pure indexing): permute nodes so each core owns 98 blocks of 128
    destination nodes with near-uniform per-source-chunk in-degree profiles
    (profile clustering).  Per (block, src-chunk) the incoming edges are laid
    out slot-major: gather position (stripe s, partition p) holds the s-th
    edge of destination node p, so every per-edge quantity is per-partition.
  * 3 launches on 8 cores (SPMD):
      1) hext1: h1 = x@W1 plus per-node attention logits -> 512B table rows
      2) msg layer 1 (+ inline h2-table build)
      3) msg layer 2 (+ log_softmax)
    Between launches the host only concatenates / reindexes device outputs.
  * Gathers: 4-queue dma_gather (int16 indices; sources split in 4 table
    chunks), 512B rows [h(128)|es(4)|ed(4)|pad] fp16.
  * Softmax: w = m * exp(leaky_relu(es_src + ed_dst)); padded slots have
    multiplicity m=0.  Aggregation: identity-lhsT matmuls accumulate
    w-weighted rows and the denominator into PSUM per block.
"""
import math
import numpy as np

import concourse.bacc as bacc
import concourse.bass as bass
import concourse.mybir as mybir
import concourse.tile as tile
from concourse.vector_clock import ScopedClock

# ----------------------------------------------------------------------------
# problem constants (hardcoded per contract)
N_NODES = 100000
N_EDGES = 1600000
D_IN = 128
H = 4
D_HID = 32
D_OUT = 32
NEG_SLOPE = 0.2

NCORES = 8
NCHUNKS = 4
ROW = 256          # fp16 elements per table row (512 B)
MAXCALL = 8        # stripes per dma_gather call (<=1024 idxs)
F32 = mybir.dt.float32
F16 = mybir.dt.float16
I16 = mybir.dt.int16

# ----------------------------------------------------------------------------
# walrus in this toolchain rejects instructions with many sync waits; move the
# excess onto same-engine nops placed before the instruction.


def _split_waits(nc, max_waits=1):
    for bb in nc.main_func.blocks:
        insts = bb.instructions
        i = 0
        while i < len(insts):
            ins = insts[i]
            si = ins.sync_info
            if si is not None and si.on_wait and len(si.on_wait) > max_waits:
                waits = list(si.on_wait)
                keep = waits[-max_waits:]
                move = waits[: len(waits) - len(keep)]
                del si.on_wait[:]
                si.on_wait.extend(keep)
                new_nops = []
                for w in move:
                    nop = nc.engines[ins.engine].nop(nofuse=True)
                    nop_ins = nop.ins
                    emitted = nc.cur_bb.bb.instructions
                    assert emitted[-1] is nop_ins
                    emitted.pop()
                    if nop_ins.sync_info is None:
                        nop_ins.sync_info = mybir.SyncInfo(on_wait=[w], on_update=[])
                    else:
                        nop_ins.sync_info.on_wait.append(w)
                    new_nops.append(nop_ins)
                insts[i:i] = new_nops
                i += len(new_nops)
            i += 1


def _drain_and_barrier_split(self, tick_clock, wait_clock):
    nc = self.nc
    drain_inst = nc.sync.drain()
    wait_clock.add_sem_waits(
        drain_inst.ins, ScopedClock({None: tick_clock.global_clock})
    )
    si = drain_inst.ins.sync_info
    if si is not None and si.on_wait and len(si.on_wait) > 1:
        waits = list(si.on_wait)
        del si.on_wait[:]
        bb = nc.cur_bb.bb
        assert bb.instructions[-1] is drain_inst.ins
        bb.instructions.pop()
        for w in waits:
            nop = nc.sync.nop(nofuse=True)
            nsi = nop.ins.sync_info
            if nsi is None:
                nop.ins.sync_info = mybir.SyncInfo(on_wait=[w], on_update=[])
            else:
                nsi.on_wait.append(w)
        bb.instructions.append(drain_inst.ins)
    nc.all_engine_barrier()
    assert self.sems is not None
    popped = nc._tile_sem_poison_stack.pop()
    assert popped is self._sem_poison
    nc.clear_and_free_semaphores(list(self.sems.allocated().values()))
    nc.all_engine_barrier()


tile.TileContext._drain_and_barrier = _drain_and_barrier_split


# ----------------------------------------------------------------------------
# host planning (pure indexing)

def build_plan(edge, n_nodes):
    src = np.asarray(edge[0], np.int64)
    dst = np.asarray(edge[1], np.int64)

    # dedup (src,dst) -> multiplicity (also provides the pad mask mechanism)
    key = dst * n_nodes + src
    uk, counts = np.unique(key, return_counts=True)
    udst = (uk // n_nodes).astype(np.int64)
    usrc = (uk % n_nodes).astype(np.int64)
    mw = counts.astype(np.float32)
    Eu = len(uk)

    chunk_map = (np.arange(n_nodes) % NCHUNKS).astype(np.int64)

    prof = np.zeros((n_nodes, NCHUNKS), np.int32)
    np.add.at(prof, (udst, chunk_map[usrc]), 1)
    deg = prof.sum(1)

    order = np.lexsort((prof[:, 2], prof[:, 1], prof[:, 0], deg))

    nblk_tot = (n_nodes + 127) // 128
    NB = (nblk_tot + NCORES - 1) // NCORES
    NPC = NB * 128
    core_nodes = -np.ones((NCORES, NPC), np.int64)
    bi = 0
    for j in range(NB):
        for c_ in range(NCORES):
            core = c_ if (j % 2 == 0) else (NCORES - 1 - c_)
            if bi >= nblk_tot:
                continue
            blk = order[bi * 128:(bi + 1) * 128]
            core_nodes[core, j * 128:j * 128 + len(blk)] = blk
            bi += 1

    # table order: chunk-major, then (core, block, partition)
    tpos = -np.ones(n_nodes, np.int64)
    chunk_bases = np.zeros(NCHUNKS + 1, np.int64)
    t = 0
    for c in range(NCHUNKS):
        chunk_bases[c] = t
        for core in range(NCORES):
            for j in range(NB):
                blk = core_nodes[core, j * 128:(j + 1) * 128]
                sel = blk[blk >= 0]
                sel = sel[chunk_map[sel] == c]
                tpos[sel] = t + np.arange(len(sel))
                t += len(sel)
    chunk_bases[NCHUNKS] = t
    assert t == n_nodes
    perm = np.empty(n_nodes, np.int64)
    perm[tpos] = np.arange(n_nodes)

    node_core = -np.ones(n_nodes, np.int64)
    node_blk = -np.ones(n_nodes, np.int64)
    node_part = -np.ones(n_nodes, np.int64)
    for core in range(NCORES):
        cn = core_nodes[core]
        pos = np.nonzero(cn >= 0)[0]
        node_core[cn[pos]] = core
        node_blk[cn[pos]] = pos // 128
        node_part[cn[pos]] = pos % 128

    ecore = node_core[udst]
    eblk = node_blk[udst]
    epart = node_part[udst]
    echunk = chunk_map[usrc]

    # slot index within (core, blk, chunk, part)
    gkey = ((ecore * NB + eblk) * NCHUNKS + echunk) * 128 + epart
    eorder = np.lexsort((gkey,))
    gk_sorted = gkey[eorder]
    grp_start = np.r_[True, gk_sorted[1:] != gk_sorted[:-1]]
    idx_in_grp = np.arange(Eu) - np.maximum.accumulate(
        np.where(grp_start, np.arange(Eu), 0))
    eslot = np.empty(Eu, np.int64)
    eslot[eorder] = idx_in_grp

    cnt = np.zeros((NCORES, NB, NCHUNKS, 128), np.int32)
    np.add.at(cnt, (ecore, eblk, echunk, epart), 1)
    S = cnt.max(axis=(0, 3)).astype(np.int64)           # [NB, NCHUNKS] compiled

    # static call schedule: per block j, per chunk c, calls of <= MAXCALL stripes
    calls = []   # (j, c, ns, iw_off, sc_off)  sc_off = stripe offset within block
    iw = 0
    block_ts = S.sum(1)                                  # stripes per block
    for j in range(NB):
        sc = 0
        for c in range(NCHUNKS):
            s_cj = int(S[j, c])
            k = 0
            while k < s_cj:
                ns = min(MAXCALL, s_cj - k)
                calls.append((j, c, ns, iw, sc))
                iw += ns * 8          # int16 cols per call ( ns*128/16 )
                sc += ns
                k += ns
    IW = iw
    TOTS = int(block_ts.sum())

    # per-core device tables (padded, per-call / per-block contiguous)
    NCALLS = len(calls)
    TSMAX = int(block_ts.max()) if NB else 1
    idx_tab = np.zeros((NCORES, NCALLS, 128, MAXCALL * 8), np.int16)
    m_tab = np.zeros((NCORES, NB, 128, TSMAX), np.float32)
    gs_of_block = np.zeros(NB + 1, np.int64)
    gs_of_block[1:] = np.cumsum(block_ts)

    # slot-major flat edge arrays
    src_tpos = tpos[usrc]
    for core in range(NCORES):
        esel = np.nonzero(ecore == core)[0]
        if len(esel) == 0:
            continue
        ej, ec, ep, es_ = eblk[esel], echunk[esel], epart[esel], eslot[esel]
        # m table: global stripe = gs_of_block[j] + (chunk stripe base) + slot
        chunk_s_base = np.zeros((NB, NCHUNKS), np.int64)
        chunk_s_base[:, 1:] = np.cumsum(S, axis=1)[:, :-1]
        bstripe = chunk_s_base[ej, ec] + es_
        m_tab[core, ej, ep, bstripe] = mw[esel]
        # idx values per call
        val = (src_tpos[esel] - chunk_bases[ec]).astype(np.int64)
        # per-call local position: stripe-within-call * 128 + part
        # build flat [S_cj*128] arrays per (j,c) then slice per call
        flat_pos_in_chunkgrp = es_ * 128 + ep
        keyjc = ej * NCHUNKS + ec
        ordjc = np.lexsort((flat_pos_in_chunkgrp, keyjc))
        # iterate calls
        ptr = 0
        esel_o = esel[ordjc]
        kj, kc = ej[ordjc], ec[ordjc]
        kpos = flat_pos_in_chunkgrp[ordjc]
        kval = val[ordjc]
        # group boundaries per (j,c)
        bnd = np.r_[0, np.nonzero((kj[1:] != kj[:-1]) | (kc[1:] != kc[:-1]))[0] + 1, len(kj)]
        grp_map = {}
        for b in range(len(bnd) - 1):
            lo = bnd[b]
            grp_map[(int(kj[lo]), int(kc[lo]))] = (bnd[b], bnd[b + 1])
        for ci_, (j, c, ns, iwo, sco) in enumerate(calls):
            lo_hi = grp_map.get((j, c))
            flat = np.zeros(ns * 128, np.int64)
            if lo_hi is not None:
                lo, hi = lo_hi
                # positions within this call's stripe range
                sbase = (sco - int(np.sum(S[j, :c]))) * 128  # call start within group
                p0, p1 = sbase, sbase + ns * 128
                seg = slice(lo + np.searchsorted(kpos[lo:hi], p0),
                            lo + np.searchsorted(kpos[lo:hi], p1))
                flat[kpos[seg] - p0] = kval[seg]
            wrap = flat.reshape(ns * 8, 16).T.astype(np.int16)  # [16, ns*8]
            idx_tab[core, ci_, :, :ns * 8] = np.tile(wrap, (8, 1))

    return dict(
        perm=perm, tpos=tpos, core_nodes=core_nodes, chunk_bases=chunk_bases,
        NB=NB, NPC=NPC, S=S, calls=calls, IW=IW, TOTS=TOTS, TSMAX=TSMAX,
        NCALLS=NCALLS, idx_tab=idx_tab, m_tab=m_tab, gs_of_block=gs_of_block,
        n_nodes=n_nodes,
    )


# ----------------------------------------------------------------------------
# bass builders

def build_hext(seg_len):
    """Launch 1: per core computes table rows for `seg_len` nodes.

    inputs : xT [128, seg_len] fp16, Wt [128,128] fp16,
             as_rep [128,128] fp32, ad_rep [128,128] fp32
    output : hx [seg_len, ROW] fp16  rows = [h(128) | es(4) | ed(4) | junk]
    """
    nc = bacc.Bacc("TRN2", num_swdge_queues=4)
    xT = nc.dram_tensor("xT", [128, seg_len], F16, kind="ExternalInput")
    Wt = nc.dram_tensor("Wt", [128, 128], F16, kind="ExternalInput")
    as_rep = nc.dram_tensor("as_rep", [128, 128], F32, kind="ExternalInput")
    ad_rep = nc.dram_tensor("ad_rep", [128, 128], F32, kind="ExternalInput")
    hx = nc.dram_tensor("hx", [seg_len, ROW], F16, kind="ExternalOutput")

    ntiles = (seg_len + 127) // 128
    with tile.TileContext(nc) as tc:
        with (
            tc.tile_pool(name="consts", bufs=1) as cpool,
            tc.tile_pool(name="work", bufs=4) as pool,
            tc.tile_pool(name="ps", bufs=2, space="PSUM") as pp,
        ):
            wt = cpool.tile([128, 128], F16)
            nc.sync.dma_start(out=wt[:], in_=Wt[:])
            asr = cpool.tile([128, 128], F32)
            nc.sync.dma_start(out=asr[:], in_=as_rep[:])
            adr = cpool.tile([128, 128], F32)
            nc.sync.dma_start(out=adr[:], in_=ad_rep[:])
            for t in range(ntiles):
                nt = min(128, seg_len - t * 128)
                xt = pool.tile([128, 128], F16, tag="xt")
                nc.sync.dma_start(out=xt[:, :nt], in_=xT[:, t * 128:t * 128 + nt])
                ph = pp.tile([128, 128], F32)
                nc.tensor.matmul(ph[:nt, :], lhsT=xt[:, :nt], rhs=wt[:],
                                 start=True, stop=True)
                row = pool.tile([128, ROW], F16, tag="row")
                nc.vector.memset(row[:], 0.0)
                nc.vector.tensor_copy(row[:nt, 0:128], ph[:nt, :])
                scr = pool.tile([128, 32], F32, tag="scr")
                for h in range(H):
                    nc.vector.scalar_tensor_tensor(
                        out=scr[:nt, :], in0=ph[:nt, h * 32:(h + 1) * 32],
                        scalar=1.0, in1=asr[:nt, h * 32:(h + 1) * 32],
                        op0=mybir.AluOpType.mult, op1=mybir.AluOpType.mult,
                        accum_out=row[:nt, 128 + h:129 + h])
                for h in range(H):
                    nc.vector.scalar_tensor_tensor(
                        out=scr[:nt, :], in0=ph[:nt, h * 32:(h + 1) * 32],
                        scalar=1.0, in1=adr[:nt, h * 32:(h + 1) * 32],
                        op0=mybir.AluOpType.mult, op1=mybir.AluOpType.mult,
                        accum_out=row[:nt, 132 + h:133 + h])
                nc.sync.dma_start(out=hx[t * 128:t * 128 + nt, :], in_=row[:nt, :])
    nc.compile()
    _split_waits(nc, max_waits=1)
    return nc


def build_msg(plan, n_nodes, layer2):
    """Launch 2/3: slot-major message passing for one layer on each core.

    inputs : tab [n_nodes, ROW] fp16, idxs [128, IW] int16, ms [128, TOTS] fp16,
             eds [NB*128, 4] fp32, btile [128,128] fp32, ident [128,128] fp16,
             (layer1 only) W2t [128,128] fp16, a2s_rep/a2d_rep [128,128] fp32
    output : layer1: hx2 [NPC, ROW] fp16 ; layer2: outp [NPC, 128] fp32
    """
    NB, S, calls = plan["NB"], plan["S"], plan["calls"]
    NPC, TSMAX, NCALLS = plan["NPC"], plan["TSMAX"], plan["NCALLS"]
    cb = plan["chunk_bases"]
    gs_of_block = plan["gs_of_block"]

    nc = bacc.Bacc("TRN2", num_swdge_queues=4)
    tab = nc.dram_tensor("tab", [n_nodes, ROW], F16, kind="ExternalInput")
    idxs = nc.dram_tensor("idxs", [NCALLS, 128, MAXCALL * 8], I16, kind="ExternalInput")
    ms = nc.dram_tensor("ms", [NB, 128, TSMAX], F32, kind="ExternalInput")
    eds = nc.dram_tensor("eds", [NB * 128, 4], F16, kind="ExternalInput")
    btile = nc.dram_tensor("btile", [128, 128], F32, kind="ExternalInput")
    identt = nc.dram_tensor("ident", [128, 128], F16, kind="ExternalInput")
    if not layer2:
        W2t = nc.dram_tensor("W2t", [128, 128], F16, kind="ExternalInput")
        a2s = nc.dram_tensor("a2s_rep", [128, 128], F32, kind="ExternalInput")
        a2d = nc.dram_tensor("a2d_rep", [128, 128], F32, kind="ExternalInput")
        hx2 = nc.dram_tensor("hx2", [NPC, ROW], F16, kind="ExternalOutput")
    else:
        outp = nc.dram_tensor("outp", [NPC, 128], F32, kind="ExternalOutput")

    A = mybir.AluOpType
    qn = 0
    with tile.TileContext(nc) as tc:
        with (
            tc.tile_pool(name="consts", bufs=1) as cpool,
            tc.tile_pool(name="gath", bufs=6) as gp,
            tc.tile_pool(name="ip", bufs=6) as ipool,
            tc.tile_pool(name="wp", bufs=6) as wp,
            tc.tile_pool(name="msgp", bufs=6) as mp,
            tc.tile_pool(name="blkp", bufs=3) as bp,
            tc.tile_pool(name="finp", bufs=3) as fp_,
            tc.tile_pool(name="psb", bufs=2, space="PSUM") as ppb,
            tc.tile_pool(name="psx", bufs=2, space="PSUM") as ppx,
        ):
            ident = cpool.tile([128, 128], F16)
            nc.sync.dma_start(out=ident[:], in_=identt[:])
            bt = cpool.tile([128, 128], F32)
            nc.sync.dma_start(out=bt[:], in_=btile[:])
            if not layer2:
                w2 = cpool.tile([128, 128], F16)
                nc.sync.dma_start(out=w2[:], in_=W2t[:])
                a2sr = cpool.tile([128, 128], F32)
                nc.sync.dma_start(out=a2sr[:], in_=a2s[:])
                a2dr = cpool.tile([128, 128], F32)
                nc.sync.dma_start(out=a2dr[:], in_=a2d[:])
                # device-side pack: w2e = [W2 | W2@blockdiag(a2s) | W2@blockdiag(a2d)]
                w2e = cpool.tile([128, 136], F16)
                nc.vector.tensor_copy(w2e[:, 0:128], w2[:])
                pscr = cpool.tile([128, 32], F32)
                for h in range(H):
                    nc.vector.scalar_tensor_tensor(
                        out=pscr[:], in0=w2[:, h * 32:(h + 1) * 32], scalar=1.0,
                        in1=a2sr[:, h * 32:(h + 1) * 32],
                        op0=A.mult, op1=A.mult,
                        accum_out=w2e[:, 128 + h:129 + h])
                for h in range(H):
                    nc.vector.scalar_tensor_tensor(
                        out=pscr[:], in0=w2[:, h * 32:(h + 1) * 32], scalar=1.0,
                        in1=a2dr[:, h * 32:(h + 1) * 32],
                        op0=A.mult, op1=A.mult,
                        accum_out=w2e[:, 132 + h:133 + h])

            ci = 0  # call index
            for j in range(NB):
                TS = int(S[j].sum())
                if TS == 0:
                    continue
                mt = bp.tile([128, TSMAX], F32, tag="mt")
                nc.sync.dma_start(out=mt[:, :TS], in_=ms[j, :, :TS])
                edt = bp.tile([128, 4], F16, tag="edt")
                nc.sync.dma_start(out=edt[:], in_=eds[j * 128:(j + 1) * 128, :])
                pb = ppb.tile([128, 132], F32)
                sc_done = 0
                while ci < len(calls) and calls[ci][0] == j:
                    _, c, ns, iwo, sco = calls[ci]
                    it = ipool.tile([128, MAXCALL * 8], I16, tag="it")
                    nc.sync.dma_start(out=it[:, :ns * 8],
                                      in_=idxs[ci, :, :ns * 8])
                    gt = gp.tile([128, MAXCALL * ROW], F16, tag="gt")
                    nc.gpsimd.dma_gather(
                        gt[:, :ns * ROW].rearrange("p (k e) -> p k e", e=ROW),
                        tab[int(cb[c]):int(cb[c + 1]), :],
                        it[:, :ns * 8], ns * 128, ns * 128, ROW,
                        queue_num=qn % 4)
                    qn += 1
                    # w path
                    wt_ = wp.tile([128, MAXCALL * 4], F32, tag="wt")
                    es_v = gt[:, :ns * ROW].rearrange("p (k e) -> p k e", e=ROW)[:, :, 128:132]
                    _edt = edt[:]
                    nc.vector.tensor_tensor(
                        out=wt_[:, :ns * 4].rearrange("p (k e) -> p k e", e=4),
                        in0=es_v,
                        in1=bass.AP(_edt.tensor, _edt.offset,
                                    [_edt.ap[0], [0, ns], [1, 4]]),
                        op=A.add)
                    nc.vector.scalar_tensor_tensor(
                        out=wt_[:, :ns * 4], in0=wt_[:, :ns * 4], scalar=NEG_SLOPE,
                        in1=wt_[:, :ns * 4], op0=A.mult, op1=A.max)
                    nc.scalar.activation(wt_[:, :ns * 4], wt_[:, :ns * 4],
                                         mybir.ActivationFunctionType.Exp)
                    # msg = [h*w | w]; w lands directly in cols 128:132
                    msg = mp.tile([128, MAXCALL * 132], F16, tag="msg")
                    msg_v = msg[:, :ns * 132].rearrange("p (k e) -> p k e", e=132)
                    mv = mt[:, sc_done:sc_done + ns]
                    nc.vector.tensor_tensor(
                        out=msg_v[:, :, 128:132],
                        in0=wt_[:, :ns * 4].rearrange("p (k e) -> p k e", e=4),
                        in1=bass.AP(mv.tensor, mv.offset, [mv.ap[0], [mv.ap[1][0], ns], [0, 4]]),
                        op=A.mult)
                    h_v = gt[:, :ns * ROW].rearrange("p (k e) -> p k e", e=ROW)[:, :, 0:128]
                    wv = msg_v[:, :, 128:132]
                    nc.vector.tensor_tensor(
                        out=msg_v[:, :, 0:128].rearrange("p k (h d) -> p k h d", d=32),
                        in0=h_v.rearrange("p k (h d) -> p k h d", d=32),
                        in1=bass.AP(wv.tensor, wv.offset,
                                    [wv.ap[0], [132, ns], [1, 4], [0, 32]]),
                        op=A.mult)
                    for s in range(ns):
                        nc.tensor.matmul(
                            pb[:], lhsT=ident[:], rhs=msg[:, s * 132:(s + 1) * 132],
                            start=(sc_done + s == 0), stop=(sc_done + s == TS - 1))
                    sc_done += ns
                    ci += 1
                # finalize block j
                den = fp_.tile([128, 4], F32, tag="den")
                nc.vector.tensor_scalar_add(den[:], pb[:, 128:132], 1e-20)
                nc.vector.reciprocal(den[:], den[:])
                t1 = fp_.tile([128, 128], F32, tag="t1")
                nc.vector.tensor_tensor(
                    out=t1[:].rearrange("p (h d) -> p h d", d=32),
                    in0=pb[:, 0:128].rearrange("p (h d) -> p h d", d=32),
                    in1=bass.AP(den.tensor, den.offset, [den.ap[0], [1, 4], [0, 32]]),
                    op=A.mult)
                nc.vector.tensor_tensor(out=t1[:], in0=t1[:], in1=bt[:], op=A.add)
                if not layer2:
                    x2 = fp_.tile([128, 128], F16, tag="x2")
                    nc.vector.tensor_scalar_max(x2[:], t1[:], 0.0)
                    px = ppx.tile([128, 128], F16)
                    nc.tensor.transpose(px[:], x2[:], ident[:])
                    x2t = fp_.tile([128, 128], F16, tag="x2t")
                    nc.vector.tensor_copy(x2t[:], px[:])
                    ph2 = ppx.tile([128, 136], F32)
                    nc.tensor.matmul(ph2[:], lhsT=x2t[:], rhs=w2e[:],
                                     start=True, stop=True)
                    row = fp_.tile([128, ROW], F16, tag="row")
                    nc.vector.memset(row[:], 0.0)
                    nc.vector.tensor_copy(row[:, 0:136], ph2[:])
                    nc.sync.dma_start(out=hx2[j * 128:(j + 1) * 128, :], in_=row[:])
                else:
                    et = fp_.tile([128, 128], F32, tag="et")
                    nc.scalar.activation(et[:], t1[:],
                                         mybir.ActivationFunctionType.Exp)
                    ssum = fp_.tile([128, 1], F32, tag="ssum")
                    nc.vector.tensor_reduce(ssum[:], et[:],
                                            axis=mybir.AxisListType.X, op=A.add)
                    nc.scalar.activation(ssum[:], ssum[:],
                                         mybir.ActivationFunctionType.Ln)
                    nc.vector.tensor_scalar_mul(ssum[:], ssum[:], -1.0)
                    to = fp_.tile([128, 128], F32, tag="to")
                    nc.scalar.activation(to[:], t1[:],
                                         mybir.ActivationFunctionType.Identity,
                                         bias=ssum[:, 0:1])
                    nc.sync.dma_start(out=outp[j * 128:(j + 1) * 128, :], in_=to[:])
    nc.compile()
    _split_waits(nc, max_waits=1)
    return nc


# ----------------------------------------------------------------------------
# runner

def _rep_heads(a):
    """[H, d] -> [128, H*d] fp32 replicated across partitions."""
    return np.tile(a.reshape(1, -1).astype(np.float32), (128, 1))


def _run(nc, in_maps):
    from concourse.bass_utils import run_bass_kernel_spmd
    return run_bass_kernel_spmd(nc, in_maps, core_ids=list(range(NCORES)),
                                trace=False).results


def run_pipeline(inputs, n_nodes, run=_run):
    edge = np.asarray(inputs["edge"])
    x = np.asarray(inputs["features"], np.float32)
    W1 = np.asarray(inputs["W1"], np.float32)
    a1s = np.asarray(inputs["a1_src"], np.float32)
    a1d = np.asarray(inputs["a1_dst"], np.float32)
    b1 = np.asarray(inputs["b1"], np.float32)
    W2 = np.asarray(inputs["W2"], np.float32)
    a2s = np.asarray(inputs["a2_src"], np.float32)
    a2d = np.asarray(inputs["a2_dst"], np.float32)
    b2 = np.asarray(inputs["b2"], np.float32)

    plan = build_plan(edge, n_nodes)
    NB, NPC = plan["NB"], plan["NPC"]
    perm, tpos = plan["perm"], plan["tpos"]
    core_nodes = plan["core_nodes"]

    # ---- launch 1: hext1
    seg = n_nodes // NCORES
    assert seg * NCORES == n_nodes
    nc1 = build_hext(seg)
    in1 = []
    for core in range(NCORES):
        seg_nodes = perm[core * seg:(core + 1) * seg]
        xT = np.ascontiguousarray(x[seg_nodes].astype(np.float16).T)
        in1.append({
            "xT": xT, "Wt": W1.astype(np.float16),
            "as_rep": _rep_heads(a1s), "ad_rep": _rep_heads(a1d),
        })
    res1 = run(nc1, in1)
    tab1 = np.concatenate([np.asarray(res1[c]["hx"]) for c in range(NCORES)], 0)

    # ed per (core, block-partition) from table rows
    def ed_for(tab):
        eds = np.zeros((NCORES, NB * 128, 4), np.float16)
        for core in range(NCORES):
            cn = core_nodes[core]
            vm = cn >= 0
            eds[core][vm] = tab[tpos[cn[vm]], 132:136]
        return eds

    eds1 = ed_for(tab1)

    # ---- launch 2: layer-1 message passing + inline h2 table rows
    nc2 = build_msg(plan, n_nodes, layer2=False)
    ident = np.eye(128, dtype=np.float16)
    in2 = []
    for core in range(NCORES):
        in2.append({
            "tab": tab1, "idxs": plan["idx_tab"][core], "ms": plan["m_tab"][core],
            "eds": eds1[core], "btile": np.tile(b1.reshape(1, -1), (128, 1)).astype(np.float32),
            "ident": ident, "W2t": W2.astype(np.float16),
            "a2s_rep": _rep_heads(a2s), "a2d_rep": _rep_heads(a2d),
        })
    res2 = run(nc2, in2)

    # assemble layer-2 table (block-order rows -> table order)
    tab2 = np.zeros((n_nodes, ROW), np.float16)
    for core in range(NCORES):
        cn = core_nodes[core]
        vm = cn >= 0
        tab2[tpos[cn[vm]]] = np.asarray(res2[core]["hx2"])[vm]
    eds2 = ed_for(tab2)

    # ---- launch 3: layer-2 message passing + log_softmax
    nc3 = build_msg(plan, n_nodes, layer2=True)
    in3 = []
    for core in range(NCORES):
        in3.append({
            "tab": tab2, "idxs": plan["idx_tab"][core], "ms": plan["m_tab"][core],
            "eds": eds2[core], "btile": np.tile(b2.reshape(1, -1), (128, 1)).astype(np.float32),
            "ident": ident,
        })
    res3 = run(nc3, in3)

    out = np.zeros((n_nodes, H * D_OUT), np.float32)
    for core in range(NCORES):
        cn = core_nodes[core]
        vm = cn >= 0
        out[cn[vm]] = np.asarray(res3[core]["outp"])[vm]
    return out


def kernel(**inputs):
    return run_pipeline(inputs, N_NODES).astype(np.float32)
